# revision 6
# baseline (speedup 1.0000x reference)
"""HAN (heterogeneous graph attention) Bass/trn2 kernel for nn_HAN_34651796144563.

kernel(**inputs) takes FULL unsharded inputs and returns (out_a, out_b) f32.

Distribution (dst-partition per the sharding hint): destination nodes are split
into 8 contiguous shards, one per NeuronCore. On each core:
  phase 0   : project its node shard (h = x @ W + b) on PE, compute per-node
              dst-attention terms; AllGather shards into full bf16 node tables.
  phase 0.5 : build per-(relation, half-shard) compact gather tables (sorted
              deduped src rows) with 4 windowed int16 dma_gathers each.
  phase 1   : per 128-dst-node tile: batched dma_gathers fetch edge payloads
              (h_src rows from the compact table, a_d rows from the local AD
              table); a_s via DVE grouped reduce; edge softmax num/den via
              one-hot matmuls accumulated in PSUM; per-tile normalize + relu.
  phase 2   : semantic attention (PE transpose + stationary Wk matmul + tanh,
              AllReduce of the mean), combine, BatchNorm (AllReduce stats).
Host does data movement / index prep only: sharding, edge grouping, compact
table index construction, padding, final concatenation.
"""
import numpy as np
import ml_dtypes

N = 100000
E = 400000
IN = 128
HID = 128
H = 8
D = HID // H
NEG_SLOPE = 0.2
BN_EPS = 1e-5
NCORES = 8
NSHARD = N // NCORES             # 12500 dst nodes per core per type

TILES = (NSHARD + 127) // 128    # 98 output tiles (last covers 84 rows)
CPT = 5                          # chunks of 128 edges per tile
HALF_TILES = 49
HALVES = 2
CH_HALF = HALF_TILES * CPT       # 245 chunks per half
SLOTS_HALF = CH_HALF * 128       # 31360 edge slots per half
NWIN = 4
WROWS = N // NWIN                # 25000 table rows per builder window
BCAP = 6400                      # compact rows per (relation, half, window)
CROWS = NWIN * BCAP              # 25600 compact rows per (relation, half)
GTILES = 7                       # tiles per gather group
GCH = GTILES * CPT               # 35 chunks per gather group
NGROUPS = HALF_TILES // GTILES   # 7 groups per half

RELS = ("ba", "aa", "ab", "bb")
REL_SRC = {"ab": "a", "aa": "a", "ba": "b", "bb": "b"}
DST_RELS = {"a": ("ba", "aa"), "b": ("ab", "bb")}
REL_COL = {r: i for i, r in enumerate(RELS)}

_COMPILED = {}
STOP_AFTER = 99   # 0=phase0 1=+builders 2=+phase1(gather only) 3=+attn-math 99=full
REP0 = 1          # phase-0 repetitions (ablation)
REPB = 1          # builder repetitions
REP1 = 1          # phase-1 repetitions
DO_ALLGATHER = True   # ablation: AllGather of h shards
DO_P2_COLL = True     # ablation: phase-2 AllReduces


# ============================ host-side preparation ==========================

def _wrap_idx(flat):
    """int array [n] (n % 16 == 0) -> [128, n//16] int16 wrapped+replicated."""
    n = flat.shape[0]
    i = np.arange(n)
    w = np.zeros((16, n // 16), np.int16)
    w[i % 16, i // 16] = flat.astype(np.int16)
    return np.tile(w, (8, 1))


def _prep_relation(src, dst, core):
    lo = core * NSHARD
    sel = (dst >= lo) & (dst < lo + NSHARD)
    s = src[sel].astype(np.int64)
    dl = (dst[sel] - lo).astype(np.int64)
    tile = dl // 128

    order = np.lexsort((dl, tile))
    s, dl, tile = s[order], dl[order], tile[order]

    slot_src = np.zeros((HALVES, SLOTS_HALF), np.int64)
    slot_dst = np.full((HALVES, SLOTS_HALF), 255, np.int64)
    slot_ad = np.zeros((HALVES, SLOTS_HALF), np.int64)
    slot_valid = np.zeros((HALVES, SLOTS_HALF), bool)

    counts = np.bincount(tile, minlength=TILES)
    starts = np.concatenate([[0], np.cumsum(counts)])
    cap = CPT * 128
    for t in range(TILES):
        n_t = min(int(counts[t]), cap)      # overflow beyond cap: dropped
        half = 0 if t < HALF_TILES else 1
        base = (t - half * HALF_TILES) * cap
        a = starts[t]
        slot_src[half, base:base + n_t] = s[a:a + n_t]
        slot_dst[half, base:base + n_t] = dl[a:a + n_t] - t * 128
        slot_ad[half, base:base + n_t] = dl[a:a + n_t]
        slot_valid[half, base:base + n_t] = True

    builder_idx = np.zeros((HALVES, NWIN, BCAP), np.int64)
    eidx = np.zeros((HALVES, SLOTS_HALF), np.int64)
    for hlf in range(HALVES):
        v = slot_valid[hlf]
        su = np.unique(slot_src[hlf][v])
        # compact position for every node id (dense lookup array)
        pos = np.zeros(N, np.int64)
        ok = np.zeros(N, bool)
        for w in range(NWIN):
            wsel = su[(su >= w * WROWS) & (su < (w + 1) * WROWS)]
            if wsel.shape[0] > BCAP:
                wsel = wsel[:BCAP]
            builder_idx[hlf, w, :wsel.shape[0]] = wsel - w * WROWS
            pos[wsel] = w * BCAP + np.arange(wsel.shape[0])
            ok[wsel] = True
        good = v & ok[slot_src[hlf]]
        eidx[hlf][good] = pos[slot_src[hlf][good]]
        # edges whose src row was dropped from the compact table: disable
        slot_dst[hlf][v & ~ok[slot_src[hlf]]] = 255
    return eidx, slot_dst, slot_ad, builder_idx


def _prep_core_inputs(inp, core):
    bf16 = ml_dtypes.bfloat16
    out = {}
    lo = core * NSHARD
    out["xT_a"] = np.ascontiguousarray(inp["x_a"][lo:lo + NSHARD].T)
    out["xT_b"] = np.ascontiguousarray(inp["x_b"][lo:lo + NSHARD].T)
    for r in RELS:
        ei = inp["ei_" + r]
        eidx, dloc, adix, bix = _prep_relation(ei[0], ei[1], core)
        eg = np.zeros((HALVES, NGROUPS, 128, GCH * 8), np.int16)
        ag = np.zeros((HALVES, NGROUPS, 128, GCH * 8), np.int16)
        for hlf in range(HALVES):
            for g in range(NGROUPS):
                sl = slice(g * GCH * 128, (g + 1) * GCH * 128)
                eg[hlf, g] = _wrap_idx(eidx[hlf][sl])
                ag[hlf, g] = _wrap_idx(adix[hlf][sl])
        out[f"eidx_{r}"] = eg
        out[f"adix_{r}"] = ag
        dst_cm = dloc.reshape(HALVES, CH_HALF, 128).transpose(0, 2, 1)
        out[f"dloc_{r}"] = np.ascontiguousarray(dst_cm).astype(bf16)
        bb = np.zeros((HALVES, NWIN, 128, BCAP // 16), np.int16)
        for hlf in range(HALVES):
            for w in range(NWIN):
                bb[hlf, w] = _wrap_idx(bix[hlf, w])
        out[f"bidx_{r}"] = bb
    return out


def _const_inputs(inp):
    bf16 = ml_dtypes.bfloat16
    f32 = np.float32
    c = {}
    c["W_a"] = np.asarray(inp["W_a"], f32)
    c["W_b"] = np.asarray(inp["W_b"], f32)
    c["b_a"] = np.broadcast_to(
        np.asarray(inp["b_a"], f32).astype(bf16), (128, HID)).copy()
    c["b_b"] = np.broadcast_to(
        np.asarray(inp["b_b"], f32).astype(bf16), (128, HID)).copy()
    for r in RELS:
        s = np.asarray(inp[f"att_{r}_s"], f32).reshape(-1)
        d = np.asarray(inp[f"att_{r}_d"], f32).reshape(-1)
        c[f"attS_{r}"] = np.broadcast_to(s.astype(bf16), (128, HID)).copy()
        c[f"attD_{r}"] = np.broadcast_to(d.astype(bf16), (128, HID)).copy()
    c["Wk"] = np.asarray(inp["Wk"], f32).astype(bf16)
    c["bk"] = np.asarray(inp["bk"], f32).reshape(HID, 1)
    c["q"] = np.asarray(inp["q"], f32).reshape(HID, 1)
    c["gamma"] = np.asarray(inp["gamma"], f32).reshape(HID, 1)
    c["beta"] = np.asarray(inp["beta"], f32).reshape(HID, 1)
    c["iota"] = np.broadcast_to(
        np.arange(128, dtype=f32).astype(bf16), (128, 128)).copy()
    c["ident"] = np.eye(128, dtype=f32).astype(bf16)
    c["ones"] = np.ones((128, 1), f32).astype(bf16)
    return c


# ============================== device program ==============================

def _build_program():
    import concourse.bacc as bacc
    import concourse.bass as bass
    import concourse.mybir as mybir
    import concourse.tile as tile
    from contextlib import ExitStack

    f32 = mybir.dt.float32
    bf16 = mybir.dt.bfloat16
    i16 = mybir.dt.int16
    AF = mybir.ActivationFunctionType
    OP = mybir.AluOpType
    AX = mybir.AxisListType

    nc = bacc.Bacc(num_devices=NCORES)
    groups = [list(range(NCORES))]

    def din(name, shape, dt):
        return nc.dram_tensor(name, shape, dt, kind="ExternalInput")

    xT = {t: din(f"xT_{t}", [128, NSHARD], f32) for t in "ab"}
    W = {t: din(f"W_{t}", [IN, HID], f32) for t in "ab"}
    bB = {t: din(f"b_{t}", [128, HID], bf16) for t in "ab"}
    attS = {r: din(f"attS_{r}", [128, HID], bf16) for r in RELS}
    attD = {r: din(f"attD_{r}", [128, HID], bf16) for r in RELS}
    Wk_in = din("Wk", [HID, HID], bf16)
    bk_in = din("bk", [HID, 1], f32)
    q_in = din("q", [HID, 1], f32)
    gam_in = din("gamma", [HID, 1], f32)
    bet_in = din("beta", [HID, 1], f32)
    iota_in = din("iota", [128, 128], bf16)
    ident_in = din("ident", [128, 128], bf16)
    ones_in = din("ones", [128, 1], bf16)
    eidx = {r: din(f"eidx_{r}", [HALVES, NGROUPS, 128, GCH * 8], i16)
            for r in RELS}
    adix = {r: din(f"adix_{r}", [HALVES, NGROUPS, 128, GCH * 8], i16)
            for r in RELS}
    dloc_in = {r: din(f"dloc_{r}", [HALVES, 128, CH_HALF], bf16) for r in RELS}
    bidx = {r: din(f"bidx_{r}", [HALVES, NWIN, 128, BCAP // 16], i16)
            for r in RELS}

    out_d = {t: nc.dram_tensor(f"out_{t}", [NSHARD, HID], f32,
                               kind="ExternalOutput") for t in "ab"}
    dbg_o = nc.dram_tensor("dbg_o", [2, 128, TILES * 128], mybir.dt.bfloat16,
                           kind="ExternalOutput")
    dbg_sem = nc.dram_tensor("dbg_sem", [128, 4], f32, kind="ExternalOutput")
    dbg_at = nc.dram_tensor("dbg_at", [2, 2], f32, kind="ExternalOutput")
    dbg_bn = nc.dram_tensor("dbg_bn", [2, 128, 2], f32, kind="ExternalOutput")

    hsh = {t: nc.dram_tensor(f"hsh_{t}", [NSHARD, HID], bf16) for t in "ab"}
    Hfull = {t: nc.dram_tensor(f"H_{t}", [N, HID], bf16) for t in "ab"}
    AD = {t: nc.dram_tensor(f"AD_{t}", [NSHARD, 128], bf16) for t in "ab"}
    compact = {r: nc.dram_tensor(f"cmp_{r}", [HALVES, CROWS, HID], bf16)
               for r in RELS}
    ar_sem_in = nc.dram_tensor("ar_sem_in", [128, 4], f32)
    ar_sem_out = nc.dram_tensor("ar_sem_out", [128, 4], f32,
                                addr_space="Shared")
    ar_bn_in = nc.dram_tensor("ar_bn_in", [128, 4], f32)
    ar_bn_out = nc.dram_tensor("ar_bn_out", [128, 4], f32, addr_space="Shared")

    def bc_heads(ap):
        """[128, H] AP -> [128, H, D] with 0-stride inner dim."""
        return bass.AP(tensor=ap.tensor, offset=ap.offset,
                       ap=[ap.ap[0], list(ap.ap[1]), [0, D]])

    with tile.TileContext(nc) as tc, ExitStack() as ctx:
        consts = ctx.enter_context(tc.tile_pool(name="consts", bufs=1))
        cW = {t: consts.tile([IN, HID], f32, tag=f"cw{t}", name=f"cw{t}") for t in "ab"}
        cB = {t: consts.tile([128, HID], bf16, tag=f"cb{t}", name=f"cb{t}") for t in "ab"}
        cAS = {r: consts.tile([128, HID], bf16, tag=f"cas{r}", name=f"cas{r}") for r in RELS}
        cAD = {r: consts.tile([128, HID], bf16, tag=f"cad{r}", name=f"cad{r}") for r in RELS}
        cDL = {r: consts.tile([128, HALVES, CH_HALF], bf16, tag=f"cdl{r}", name=f"cdl{r}")
               for r in RELS}
        cIota = consts.tile([128, 128], bf16, tag="ciota", name="ciota")
        cId = consts.tile([128, 128], bf16, tag="cident", name="cident")
        cWk = consts.tile([HID, HID], bf16, tag="cwk", name="cwk")
        cbk = consts.tile([HID, 1], f32, tag="cbk", name="cbk")
        cq = consts.tile([HID, 1], f32, tag="cq", name="cq")
        cgam = consts.tile([HID, 1], f32, tag="cgam", name="cgam")
        cbet = consts.tile([HID, 1], f32, tag="cbet", name="cbet")
        cOnes = consts.tile([128, 1], bf16, tag="cones", name="cones")
        for t in "ab":
            nc.sync.dma_start(cW[t][:], W[t][:])
            nc.sync.dma_start(cB[t][:], bB[t][:])
        for r in RELS:
            nc.sync.dma_start(cAS[r][:], attS[r][:])
            nc.sync.dma_start(cAD[r][:], attD[r][:])
            nc.sync.dma_start(cDL[r][:], dloc_in[r][:].rearrange(
                "h p c -> p h c"))
        nc.sync.dma_start(cIota[:], iota_in[:])
        nc.sync.dma_start(cId[:], ident_in[:])
        nc.sync.dma_start(cWk[:], Wk_in[:])
        nc.sync.dma_start(cbk[:], bk_in[:])
        nc.sync.dma_start(cq[:], q_in[:])
        nc.sync.dma_start(cgam[:], gam_in[:])
        nc.sync.dma_start(cbet[:], bet_in[:])
        nc.sync.dma_start(cOnes[:], ones_in[:])

        obuf = consts.tile([128, 2, TILES * 128], bf16, tag="obuf", name="obuf")
        ar_sem_t = consts.tile([128, 4], f32, tag="arsem", name="arsem")
        ar_bn_t = consts.tile([128, 4], f32, tag="arbn", name="arbn")
        nc.vector.memset(ar_sem_t[:], 0.0)
        nc.vector.memset(ar_bn_t[:], 0.0)
        ceps = consts.tile([128, 1], f32, tag="ceps", name="ceps")
        nc.vector.memset(ceps[:], BN_EPS)

        # ---------------- phase 0: projections + AD tables ----------------
        with (
            tc.tile_pool(name="p0", bufs=3) as p0,
            tc.tile_pool(name="p0ps", bufs=2, space="PSUM") as p0ps,
        ):
            for _rep0 in range(REP0):
              for t in "ab":
                for nt in range(TILES):
                    a0 = nt * 128
                    n_n = min(128, NSHARD - a0)
                    xt = p0.tile([128, 128], f32, tag="xt", name="xt")
                    nc.sync.dma_start(xt[:, :n_n], xT[t][:, a0:a0 + n_n])
                    ph = p0ps.tile([128, HID], f32, tag="ph", name="ph")
                    nc.tensor.matmul(ph[:n_n, :], xt[:, :n_n],
                                     cW[t][:], start=True, stop=True)
                    ht = p0.tile([128, HID], bf16, tag="ht", name="ht")
                    nc.vector.tensor_tensor(out=ht[:n_n, :], in0=ph[:n_n, :],
                                            in1=cB[t][:n_n, :], op=OP.add)
                    nc.sync.dma_start(hsh[t][a0:a0 + n_n, :], ht[:n_n, :])
                    adt = p0.tile([128, 128], bf16, tag="adt", name="adt")
                    nc.vector.memset(adt[:], 0.0)
                    for k, r in enumerate(DST_RELS[t]):
                        pr = p0.tile([128, HID], bf16, tag="pr", name="pr")
                        nc.vector.tensor_tensor(out=pr[:n_n, :],
                                                in0=ht[:n_n, :],
                                                in1=cAD[r][:n_n, :],
                                                op=OP.mult)
                        ad8 = p0.tile([128, 8], f32, tag="ad8", name="ad8")
                        nc.vector.tensor_reduce(
                            out=ad8[:n_n, :],
                            in_=pr[:n_n, :].rearrange("p (h d) -> p h d", d=D),
                            axis=AX.X, op=OP.add)
                        nc.vector.tensor_copy(adt[:n_n, 8 * k:8 * k + 8],
                                              ad8[:n_n, :])
                    nc.sync.dma_start(AD[t][a0:a0 + n_n, :], adt[:n_n, :])
            for t in ("ab" if DO_ALLGATHER else ""):
                nc.gpsimd.collective_compute(
                    "AllGather", OP.bypass, replica_groups=groups,
                    ins=[hsh[t][:].opt()], outs=[Hfull[t][:].opt()])

        # ---------------- phase 0.5: compact table builders ----------------
        with tc.tile_pool(name="bld", bufs=2) as bld:
          for _repb in range(REPB):
            for r in (RELS if STOP_AFTER >= 1 else ()):
                st = REL_SRC[r]
                for hlf in range(HALVES):
                    for w in range(NWIN):
                        bi = bld.tile([128, BCAP // 16], i16, tag="bi", name="bi")
                        nc.sync.dma_start(bi[:], bidx[r][hlf, w, :, :])
                        bg = bld.tile([128, BCAP // 128, HID], bf16, tag="bg", name="bg")
                        nc.gpsimd.dma_gather(
                            out_ap=bg[:],
                            in_ap=Hfull[st][w * WROWS:(w + 1) * WROWS, :],
                            idxs_ap=bi[:], num_idxs=BCAP, num_idxs_reg=BCAP,
                            elem_size=HID, single_packet=False)
                        nc.gpsimd.dma_start(
                            compact[r][hlf, w * BCAP:(w + 1) * BCAP, :]
                            .rearrange("(c p) d -> p c d", p=128),
                            bg[:])

        # ---------------- phase 1 + 2 ----------------
        with (
            tc.tile_pool(name="gst", bufs=2) as gst,
            tc.tile_pool(name="wrk", bufs=3) as wrk,
            tc.tile_pool(name="sem", bufs=2) as semp,
            tc.tile_pool(name="ps1", bufs=2, space="PSUM") as ps1,
            tc.tile_pool(name="ps2", bufs=2, space="PSUM") as ps2,
            tc.tile_pool(name="ps3", bufs=1, space="PSUM") as ps3,
        ):
            for _rep1 in range(REP1):
              for dty in ("ab" if STOP_AFTER >= 2 else ""):
                dti = 0 if dty == "a" else 1
                for ri, r in enumerate(DST_RELS[dty]):
                    for hlf in range(HALVES):
                        for g in range(NGROUPS):
                            hs_st = gst.tile([128, GCH, HID], bf16, tag="hs", name="hs")
                            ad_st = gst.tile([128, GCH, 128], bf16, tag="ad", name="ad")
                            ei_t = gst.tile([128, GCH * 8], i16, tag="ei", name="ei")
                            ai_t = gst.tile([128, GCH * 8], i16, tag="ai", name="ai")
                            nc.sync.dma_start(ei_t[:], eidx[r][hlf, g, :, :])
                            nc.sync.dma_start(ai_t[:], adix[r][hlf, g, :, :])
                            nc.gpsimd.dma_gather(
                                out_ap=hs_st[:], in_ap=compact[r][hlf, :, :],
                                idxs_ap=ei_t[:], num_idxs=GCH * 128,
                                num_idxs_reg=GCH * 128, elem_size=HID,
                                single_packet=False)
                            nc.gpsimd.dma_gather(
                                out_ap=ad_st[:], in_ap=AD[dty][:, :],
                                idxs_ap=ai_t[:], num_idxs=GCH * 128,
                                num_idxs_reg=GCH * 128, elem_size=128,
                                single_packet=False)
                            for tt in range(GTILES if STOP_AFTER >= 3 else 0):
                                tl = hlf * HALF_TILES + g * GTILES + tt
                                pt = ps1.tile([128, HID + 8], f32, tag="pt", name="pt")
                                for c in range(CPT):
                                    ch = tt * CPT + c
                                    chh = g * GCH + ch
                                    hs = hs_st[:, ch, :]
                                    rhs = wrk.tile([128, HID + 8], bf16,
                                                   tag="rhs", name="rhs")
                                    prod = wrk.tile([128, HID], bf16,
                                                    tag="prod", name="prod")
                                    nc.vector.tensor_tensor(
                                        out=prod[:], in0=hs, in1=cAS[r][:],
                                        op=OP.mult)
                                    a_s = wrk.tile([128, H], f32, tag="a_s", name="a_s")
                                    nc.vector.tensor_reduce(
                                        out=a_s[:],
                                        in_=prod[:].rearrange(
                                            "p (h d) -> p h d", d=D),
                                        axis=AX.X, op=OP.add)
                                    a2 = wrk.tile([128, H], f32, tag="a2", name="a2")
                                    nc.vector.tensor_tensor(
                                        out=a2[:], in0=a_s[:],
                                        in1=ad_st[:, ch, 8 * ri:8 * ri + 8],
                                        op=OP.add)
                                    asc = wrk.tile([128, H], f32, tag="asc", name="asc")
                                    nc.vector.tensor_scalar_mul(
                                        asc[:], a2[:], NEG_SLOPE)
                                    al = wrk.tile([128, H], f32, tag="al", name="al")
                                    nc.vector.tensor_tensor(
                                        out=al[:], in0=a2[:], in1=asc[:],
                                        op=OP.max)
                                    nc.scalar.activation(
                                        rhs[:, HID:HID + 8], al[:], AF.Exp)
                                    nc.vector.tensor_tensor(
                                        out=rhs[:, 0:HID].rearrange(
                                            "p (h d) -> p h d", d=D),
                                        in0=hs.rearrange(
                                            "p (h d) -> p h d", d=D),
                                        in1=bc_heads(rhs[:, HID:HID + 8]),
                                        op=OP.mult)
                                    oh = wrk.tile([128, 128], bf16, tag="oh", name="oh")
                                    nc.vector.tensor_tensor(
                                        out=oh[:],
                                        in0=cDL[r][:, hlf, chh:chh + 1]
                                        .to_broadcast([128, 128]),
                                        in1=cIota[:], op=OP.is_equal)
                                    nc.tensor.matmul(
                                        pt[:], oh[:], rhs[:],
                                        start=(c == 0), stop=(c == CPT - 1))
                                den = wrk.tile([128, H], f32, tag="den", name="den")
                                nc.vector.tensor_scalar_add(
                                    den[:], pt[:, HID:HID + 8], 1e-16)
                                rec = wrk.tile([128, H], f32, tag="rec", name="rec")
                                nc.vector.reciprocal(rec[:], den[:])
                                ot = wrk.tile([128, HID], bf16, tag="ot", name="ot")
                                nc.vector.tensor_tensor(
                                    out=ot[:].rearrange(
                                        "p (h d) -> p h d", d=D),
                                    in0=pt[:, 0:HID].rearrange(
                                        "p (h d) -> p h d", d=D),
                                    in1=bc_heads(rec[:]), op=OP.mult)
                                nc.vector.tensor_scalar_max(
                                    obuf[:, ri, tl * 128:(tl + 1) * 128],
                                    ot[:], 0.0)

                    # semantic partial for this relation
                    if STOP_AFTER < 4:
                        continue
                    thr = semp.tile([128, TILES], f32, tag="thr", name="thr")
                    for tl in range(TILES):
                        n_n = min(128, NSHARD - tl * 128)
                        tp = ps2.tile([128, 128], bf16, tag="tp", name="tp")
                        nc.tensor.transpose(
                            out=tp[:, :n_n],
                            in_=obuf[:n_n, ri, tl * 128:tl * 128 + 128],
                            identity=cId[:n_n, :n_n])
                        oT = wrk.tile([128, 128], bf16, tag="oT", name="oT")
                        nc.vector.tensor_copy(oT[:, :n_n], tp[:, :n_n])
                        t2 = ps2.tile([128, 128], f32, tag="t2", name="t2")
                        nc.tensor.matmul(t2[:, :n_n], cWk[:], oT[:, :n_n],
                                         start=True, stop=True)
                        sc = wrk.tile([128, 128], f32, tag="sc", name="sc")
                        nc.scalar.activation(
                            sc[:, :n_n], t2[:, :n_n], AF.Tanh,
                            bias=cbk[:], accum_out=thr[:, tl:tl + 1])
                    col = REL_COL[r]
                    nc.vector.tensor_reduce(
                        out=ar_sem_t[:, col:col + 1], in_=thr[:],
                        axis=AX.X, op=OP.add)
                    if dty == "a":
                        nc.sync.dma_start(dbg_o[ri, :, :], obuf[:, ri, :])

                if dty == "a":
                    # stash o for type a? no — process type a fully below
                    pass

                # ---- per-type phase 2 runs after BOTH types' phase 1?
                # semantic needs a cross-core AllReduce; do it once per type
                # by deferring: here we only computed partials. Combination
                # happens after the AllReduce below, but obuf is reused per
                # type — so run the AllReduce + combine + BN inside the type
                # loop, with a separate AllReduce per type.
                if STOP_AFTER < 4:
                    continue
                arsi = nc.dram_tensor(f"arsi_{dty}_{_rep1}", [128, 4], f32)
                arso = nc.dram_tensor(f"arso_{dty}_{_rep1}", [128, 4], f32,
                                      addr_space="Shared")
                nc.sync.dma_start(arsi[:], ar_sem_t[:])
                if DO_P2_COLL:
                    nc.gpsimd.collective_compute(
                        "AllReduce", OP.add, replica_groups=groups,
                        ins=[arsi[:].opt()], outs=[arso[:].opt()])
                else:
                    nc.sync.dma_start(arso[:], arsi[:])
                ar_sem = semp.tile([128, 4], f32, tag="arsemr", name="arsemr")
                nc.sync.dma_start(ar_sem[:], arso[:])

                sc2 = wrk.tile([1, 2], f32, tag="sc2", name="sc2")
                for j, r in enumerate(DST_RELS[dty]):
                    col = REL_COL[r]
                    mean = wrk.tile([128, 1], f32, tag="mean", name="mean")
                    nc.vector.tensor_scalar_mul(
                        mean[:], ar_sem[:, col:col + 1], 1.0 / N)
                    sps = ps2.tile([1, 1], f32, tag="tp", name="tp")
                    nc.tensor.matmul(sps[:], mean[:], cq[:],
                                     start=True, stop=True)
                    nc.vector.tensor_copy(sc2[:, j:j + 1], sps[:])
                mx = wrk.tile([1, 1], f32, tag="mx", name="mx")
                nc.vector.tensor_reduce(out=mx[:], in_=sc2[:], axis=AX.X,
                                        op=OP.max)
                nmx = wrk.tile([1, 1], f32, tag="nmx", name="nmx")
                nc.vector.tensor_scalar_mul(nmx[:], mx[:], -1.0)
                e2 = wrk.tile([1, 2], f32, tag="e2", name="e2")
                nc.scalar.activation(e2[:], sc2[:], AF.Exp, bias=nmx[:])
                s2 = wrk.tile([1, 1], f32, tag="s2", name="s2")
                nc.vector.tensor_reduce(out=s2[:], in_=e2[:], axis=AX.X,
                                        op=OP.add)
                r2t = wrk.tile([1, 1], f32, tag="r2t", name="r2t")
                nc.vector.reciprocal(r2t[:], s2[:])
                at2 = wrk.tile([1, 2], f32, tag="at2", name="at2")
                nc.vector.tensor_tensor(out=at2[:], in0=e2[:],
                                        in1=r2t[:].to_broadcast([1, 2]),
                                        op=OP.mult)
                attn_bc = semp.tile([128, 2], f32, tag="attnbc", name="attnbc")
                nc.gpsimd.partition_broadcast(attn_bc[:], at2[:])
                nc.sync.dma_start(dbg_at[dti:dti + 1, :], at2[:])
                nc.sync.dma_start(dbg_sem[:, :], ar_sem[:])

                # combine + BN stats
                pstat = ps3.tile([128, 1], f32, tag="pstat", name="pstat")
                pstat2 = ps3.tile([128, 1], f32, tag="pstat2", name="pstat2")
                for tl in range(TILES):
                    n_n = min(128, NSHARD - tl * 128)
                    sl = slice(tl * 128, tl * 128 + 128)
                    m1 = wrk.tile([128, 128], bf16, tag="m1", name="m1")
                    nc.vector.tensor_scalar_mul(
                        m1[:n_n, :], obuf[:n_n, 0, sl], attn_bc[:n_n, 0:1])
                    m2 = wrk.tile([128, 128], bf16, tag="m2", name="m2")
                    nc.vector.tensor_scalar_mul(
                        m2[:n_n, :], obuf[:n_n, 1, sl], attn_bc[:n_n, 1:2])
                    nc.vector.tensor_tensor(out=obuf[:n_n, 0, sl],
                                            in0=m1[:n_n, :], in1=m2[:n_n, :],
                                            op=OP.add)
                    sq = wrk.tile([128, 128], bf16, tag="sq", name="sq")
                    nc.vector.tensor_tensor(out=sq[:], in0=obuf[:, 0, sl],
                                            in1=obuf[:, 0, sl], op=OP.mult)
                    nc.tensor.matmul(pstat[:], obuf[:, 0, sl], cOnes[:],
                                     start=(tl == 0), stop=(tl == TILES - 1),
                                     skip_group_check=True)
                    nc.tensor.matmul(pstat2[:], sq[:], cOnes[:],
                                     start=(tl == 0), stop=(tl == TILES - 1),
                                     skip_group_check=True)
                nc.vector.tensor_copy(ar_bn_t[:, 2 * dti:2 * dti + 1],
                                      pstat[:])
                nc.vector.tensor_copy(ar_bn_t[:, 2 * dti + 1:2 * dti + 2],
                                      pstat2[:])

                arbi = nc.dram_tensor(f"arbi_{dty}_{_rep1}", [128, 2], f32)
                arbo = nc.dram_tensor(f"arbo_{dty}_{_rep1}", [128, 2], f32,
                                      addr_space="Shared")
                nc.sync.dma_start(arbi[:], ar_bn_t[:, 2 * dti:2 * dti + 2])
                if DO_P2_COLL:
                    nc.gpsimd.collective_compute(
                        "AllReduce", OP.add, replica_groups=groups,
                        ins=[arbi[:].opt()], outs=[arbo[:].opt()])
                else:
                    nc.sync.dma_start(arbo[:], arbi[:])
                ar_bn = semp.tile([128, 2], f32, tag="arbnr", name="arbnr")
                nc.sync.dma_start(ar_bn[:], arbo[:])
                nc.sync.dma_start(dbg_bn[dti, :, :], ar_bn[:])

                mu = wrk.tile([128, 1], f32, tag="mu", name="mu")
                nc.vector.tensor_scalar_mul(mu[:], ar_bn[:, 0:1], 1.0 / N)
                ex2m = wrk.tile([128, 1], f32, tag="ex2m", name="ex2m")
                nc.vector.tensor_scalar_mul(ex2m[:], ar_bn[:, 1:2], 1.0 / N)
                mu2 = wrk.tile([128, 1], f32, tag="mu2", name="mu2")
                nc.vector.tensor_tensor(out=mu2[:], in0=mu[:], in1=mu[:],
                                        op=OP.mult)
                var = wrk.tile([128, 1], f32, tag="var", name="var")
                nc.vector.tensor_tensor(out=var[:], in0=ex2m[:], in1=mu2[:],
                                        op=OP.subtract)
                sd = wrk.tile([128, 1], f32, tag="sd", name="sd")
                nc.scalar.activation(sd[:], var[:], AF.Sqrt, bias=ceps[:])
                rsd = wrk.tile([128, 1], f32, tag="rsd", name="rsd")
                nc.vector.reciprocal(rsd[:], sd[:])
                scale = wrk.tile([128, 1], f32, tag="scale", name="scale")
                nc.vector.tensor_tensor(out=scale[:], in0=cgam[:],
                                        in1=rsd[:], op=OP.mult)
                musc = wrk.tile([128, 1], f32, tag="musc", name="musc")
                nc.vector.tensor_tensor(out=musc[:], in0=mu[:], in1=scale[:],
                                        op=OP.mult)
                shift = wrk.tile([128, 1], f32, tag="shift", name="shift")
                nc.vector.tensor_tensor(out=shift[:], in0=cbet[:],
                                        in1=musc[:], op=OP.subtract)
                scb = wrk.tile([128, 1], bf16, tag="scb", name="scb")
                nc.vector.tensor_copy(scb[:], scale[:])
                shb = wrk.tile([128, 1], bf16, tag="shb", name="shb")
                nc.vector.tensor_copy(shb[:], shift[:])
                tsc = ps2.tile([1, 128], bf16, tag="tp", name="tp")
                nc.tensor.transpose(out=tsc[:], in_=scb[:], identity=cId[:])
                tsh = ps2.tile([1, 128], bf16, tag="t2", name="t2")
                nc.tensor.transpose(out=tsh[:], in_=shb[:], identity=cId[:])
                scr = wrk.tile([1, 128], f32, tag="scr", name="scr")
                nc.vector.tensor_copy(scr[:], tsc[:])
                shr = wrk.tile([1, 128], f32, tag="shr", name="shr")
                nc.vector.tensor_copy(shr[:], tsh[:])
                scale_bc = semp.tile([128, 128], f32, tag="scbc", name="scbc")
                shift_bc = semp.tile([128, 128], f32, tag="shbc", name="shbc")
                nc.gpsimd.partition_broadcast(scale_bc[:], scr[:])
                nc.gpsimd.partition_broadcast(shift_bc[:], shr[:])
                for tl in range(TILES):
                    n_n = min(128, NSHARD - tl * 128)
                    sl = slice(tl * 128, tl * 128 + 128)
                    fin = wrk.tile([128, 128], f32, tag="fin", name="fin")
                    nc.vector.tensor_tensor(out=fin[:n_n, :],
                                            in0=obuf[:n_n, 0, sl],
                                            in1=scale_bc[:n_n, :],
                                            op=OP.mult)
                    nc.vector.tensor_tensor(out=fin[:n_n, :],
                                            in0=fin[:n_n, :],
                                            in1=shift_bc[:n_n, :],
                                            op=OP.add)
                    nc.sync.dma_start(
                        out_d[dty][tl * 128:tl * 128 + n_n, :], fin[:n_n, :])

    nc.compile()
    return nc


def _get_program():
    if "nc" not in _COMPILED:
        _COMPILED["nc"] = _build_program()
    return _COMPILED["nc"]


def kernel(x_a, x_b, ei_ab, ei_ba, ei_aa, ei_bb, W_a, b_a, W_b, b_b,
           att_ab_s, att_ab_d, att_ba_s, att_ba_d, att_aa_s, att_aa_d,
           att_bb_s, att_bb_d, Wk, bk, q, gamma, beta):
    from concourse.bass_utils import run_bass_kernel_spmd

    inp = dict(x_a=np.asarray(x_a, np.float32),
               x_b=np.asarray(x_b, np.float32),
               ei_ab=np.asarray(ei_ab), ei_ba=np.asarray(ei_ba),
               ei_aa=np.asarray(ei_aa), ei_bb=np.asarray(ei_bb),
               W_a=W_a, b_a=b_a, W_b=W_b, b_b=b_b,
               att_ab_s=att_ab_s, att_ab_d=att_ab_d,
               att_ba_s=att_ba_s, att_ba_d=att_ba_d,
               att_aa_s=att_aa_s, att_aa_d=att_aa_d,
               att_bb_s=att_bb_s, att_bb_d=att_bb_d,
               Wk=Wk, bk=bk, q=q, gamma=gamma, beta=beta)

    consts = _const_inputs(inp)
    in_maps = []
    for c in range(NCORES):
        m = dict(consts)
        m.update(_prep_core_inputs(inp, c))
        in_maps.append(m)

    nc = _get_program()
    res = run_bass_kernel_spmd(nc, in_maps, list(range(NCORES)))
    out_a = np.concatenate([res.results[c]["out_a"] for c in range(NCORES)], 0)
    out_b = np.concatenate([res.results[c]["out_b"] for c in range(NCORES)], 0)
    return out_a, out_b

